# revision 59
# baseline (speedup 1.0000x reference)
"""Distributed Trainium2 Bass kernel for nn_ActorGCN (GNN message passing), v4.

8 NeuronCores, SPMD, node-octile sharding.

Structure (per core, SPMD):
  MP0  edge MLP over this core's outgoing edges (src-sorted, bucket-padded,
       packed [32 feats x 4 slot-subgroups] = 128 partitions so the segment
       reduce uses the full DVE width), then v = (sums*A - Bx) folds the
       scatter-mean normalization, pad correction, +x, and *dinv into two DVE
       ops. The pair table [p, n, d] (feat p+16d) is built REPLICATED on all
       8 partition blocks by PE permute-matmuls + strided Activation copies.
  MP1  gather-partials + ReduceScatter: each Q7 core's 16-partition block c
       gathers from the LOCAL replicated table the h*dinv values of this
       core's outgoing edges whose dst lies in core c (dst-grid order,
       self-loops included), segment-reduces, permutes each block to
       canonical dst order, and a ReduceScatter(add) over cores completes
       the segment sums: rank c receives sum_g partials for its own nodes.
       Then m1 = RSout * dinv_dst, h1 = relu(Wg1^T m1 + b), and
       p2 = Wg2^T (h1*dinv) is produced replicated on 128 partitions as
       two pair-halves.
  MP2  same gather-partials + ReduceScatter on p2 (two d=2 half passes),
       t2 = RSout*dinv + bg2, h2 = relu(t2), a = relu(Wa^T h2 + ba),
       asum over real node columns only (pads sit at the end).
Host: out = (sum_c asum_c / N) @ W_o + b_o.
"""
import sys
import numpy as np

sys.path.insert(0, "/opt/trn_rl_repo")

N = 50000
E = 1600000
NCORE = 8
NSH = N // NCORE            # 6250
NSHP = NSH + 6              # 6256 (mult of 16)
F0 = 32
H = 128
F2 = 64

B0 = [4, 8, 12, 16, 20, 24, 28, 32, 36, 40, 44, 48, 56, 64, 80, 96, 128]
B1 = [1, 2, 3, 4, 5, 6, 7, 8, 10, 12, 16, 20, 24, 32, 48, 64]

GCH1 = 7424                 # MP1 gather chunk (slots), d=2
GCH2 = 6656                 # MP2 gather chunk (slots), d=2

PERM32 = np.concatenate([np.arange(0, F0, 2), np.arange(1, F0, 2)])
PERM64 = np.concatenate([np.arange(j, F2, 4) for j in range(4)])


def _bucket_vec(deg, B):
    K = np.full(deg.shape, B[-1], np.int64)
    for b in reversed(B):
        K[deg <= b] = b
    assert (deg <= K).all()
    return K


def _wrap16(vals, parts, base_part=0):
    n = len(vals)
    assert n % 16 == 0
    w = n // 16
    out = np.zeros((parts, w), np.int16)
    out[base_part:base_part + 16, :] = np.asarray(vals, np.int16).reshape(w, 16).T
    return out


def _pack(arrs, dtype):
    """arrs: list of (name, 2-D array). Returns (offmap, flat[1, L])."""
    offmap, flat, off = {}, [], 0
    for name, a in arrs:
        a = np.ascontiguousarray(np.asarray(a).astype(dtype, copy=False))
        P, n = a.shape
        offmap[name] = (off, P, n)
        flat.append(a.reshape(-1))
        off += P * n
    return offmap, np.concatenate(flat).reshape(1, -1)


def host_prep(inputs):
    import ml_dtypes
    bft = ml_dtypes.bfloat16
    x = np.asarray(inputs["x"], np.float32)
    ei = np.asarray(inputs["edge_index"])
    ea = np.asarray(inputs["edge_attr"], np.float32)
    src = ei[0].astype(np.int64)
    dst = ei[1].astype(np.int64)
    W = {k: np.asarray(inputs[k], np.float32) for k in
         ["W_e1", "b_e1", "W_e2", "b_e2", "W_g1", "b_g1", "W_g2", "b_g2",
          "W_a", "b_a", "W_o", "b_o"]}

    We2p = np.zeros((H, F0), np.float32); We2p[:, :30] = W["W_e2"]
    We2p = We2p[:, PERM32]
    be2p = np.zeros((F0,), np.float32); be2p[:30] = W["b_e2"]
    be2p = be2p[PERM32]
    Wg1p = np.zeros((F0, H), np.float32); Wg1p[:30] = W["W_g1"]
    Wg1p = Wg1p[PERM32]
    Wg2p = W["W_g2"][:, PERM64]
    bg2p = W["b_g2"][PERM64]
    Wap = W["W_a"][PERM64]
    bg2_j = np.stack([bg2p[16 * j:16 * (j + 1)] for j in range(4)], 1)  # [16,4]
    # vpad = edge-MLP output of an all-zero (padding) edge slot
    vpad = np.maximum(We2p.T @ np.maximum(W["b_e1"], 0.0) + be2p, 0.0)  # [32]

    deg_dst = np.bincount(dst, minlength=N).astype(np.float32)
    dinv_all = 1.0 / np.sqrt(deg_dst + 1.0)

    # ---------- MP0: 4-subgroup grid ----------
    per_core0 = []
    for c in range(NCORE):
        lo = c * NSH
        eidx = np.nonzero((src >= lo) & (src < lo + NSH))[0]
        s_loc = src[eidx] - lo
        order = np.argsort(s_loc, kind="stable")
        eidx = eidx[order]
        deg = np.bincount(s_loc, minlength=NSH)
        K = _bucket_vec(np.maximum(deg, 1), B0)
        per_core0.append((eidx, deg, K))
    Jb = {}
    for b in B0:
        mx = max(int((K == b).sum()) for _, _, K in per_core0)
        if mx > 0:
            Jb[b] = (((mx + 3) // 4) + 3) // 4 * 4   # ceil(mx/4) rounded to 4
    JPAD = 4
    J = sum(Jb.values()) + JPAD                       # per-sub grid length
    joff = {}
    _o = 0
    for b in B0:
        if b in Jb:
            joff[b] = _o
            _o += Jb[b]
    plan0 = []
    slots0 = 0
    for b in B0:
        if b not in Jb:
            continue
        left = Jb[b]
        jo = joff[b]
        step = max(1, 512 // b)
        while left > 0:
            m4 = min(left, step)
            plan0.append((b, m4, jo, slots0))
            slots0 += 4 * m4 * b
            jo += m4
            left -= m4

    grid_assign = []
    for c in range(NCORE):
        _, deg, K0 = per_core0[c]
        sub_of_node = np.full(NSH, -1, np.int64)
        j_of_node = np.full(NSH, -1, np.int64)
        for b in B0:
            if b not in Jb:
                continue
            nodes = np.nonzero(K0 == b)[0]
            i = np.arange(len(nodes))
            sub_of_node[nodes] = i % 4
            j_of_node[nodes] = joff[b] + i // 4
        assert (sub_of_node >= 0).all() and (j_of_node < J - JPAD).all()
        gp_of_node = sub_of_node * J + j_of_node
        grid_assign.append((sub_of_node, j_of_node, gp_of_node))

    # ---------- MP1/MP2 grids: edges (src in g) -> (dst in c) ----------
    per_cg = {}
    for c in range(NCORE):
        lo = c * NSH
        eidx = np.nonzero((dst >= lo) & (dst < lo + NSH))[0]
        g_of = src[eidx] // NSH
        for g in range(NCORE):
            e2 = eidx[g_of == g]
            d_loc = dst[e2] - lo
            s_g = src[e2] - g * NSH
            order = np.argsort(d_loc, kind="stable")
            d_loc = d_loc[order]
            s_g = s_g[order]
            sdeg = np.bincount(d_loc, minlength=NSH)
            K = _bucket_vec(np.maximum(sdeg, 1), B1)
            per_cg[(c, g)] = (d_loc, s_g, sdeg, K)

    def _round16(v):
        return (int(v) + 15) // 16 * 16
    bc1 = {b: _round16(max(int((K == b).sum())
                           for _, _, _, K in per_cg.values()))
           for b in B1}
    layout1 = [(b, bc1[b]) for b in B1 if bc1[b] > 0]
    NG1 = sum(cnt for _, cnt in layout1)
    NG1T = (NG1 + 7) // 8 * 8
    slots1 = sum(b * cnt for b, cnt in layout1)

    def _gplan(cap):
        pieces = []
        node_off = 0
        slot_off = 0
        for b, cnt in layout1:
            left = cnt
            step = max(16, (cap // b) // 16 * 16)
            while left > 0:
                m = min(left, step)
                pieces.append((b, m, node_off, slot_off))
                node_off += m
                slot_off += b * m
                left -= m
        chunks = []
        cur = []
        cur_off = 0
        cur_n = 0
        for (b, m, no, so) in pieces:
            n = b * m
            if cur and cur_n + n > cap:
                chunks.append((cur_off, cur_n, cur))
                cur = []
                cur_off = so
                cur_n = 0
            cur.append((b, m, no, so))
            cur_n += n
        if cur:
            chunks.append((cur_off, cur_n, cur))
        return chunks
    gplan1 = _gplan(GCH1)
    gplan2 = _gplan(GCH2)

    # per-(c,g) gather slot values (MP0-grid pos / canonical src) and perms
    slot_cg = {}
    perm_cg = {}
    for c in range(NCORE):
        for g in range(NCORE):
            d_loc, s_g, sdeg, K1 = per_cg[(c, g)]
            gn = np.full(NG1, -1, np.int64)
            pos = 0
            for b, cnt in layout1:
                nodes = np.nonzero(K1 == b)[0]
                gn[pos:pos + len(nodes)] = nodes
                pos += cnt
            run_b1 = np.concatenate([np.full(cnt, b) for b, cnt in layout1])
            rs1 = np.concatenate([[0], np.cumsum(run_b1)[:-1]])
            gpn = np.zeros(NSH, np.int64)
            vv = gn >= 0
            gpn[gn[vv]] = np.nonzero(vv)[0]
            firstd = np.concatenate([[0], np.cumsum(sdeg)[:-1]])
            within1 = np.arange(len(d_loc)) - firstd[d_loc]
            # MP0-grid sentinel: pad slot (zero entry) of g's grid
            slotv0 = np.full(slots1, J - 1, np.int64)
            gp_g = grid_assign[g][2]
            slotv0[rs1[gpn[d_loc]] + within1] = gp_g[s_g]
            # canonical sentinel: pad column (zero p2 entry)
            slotv2 = np.full(slots1, NSH, np.int64)
            slotv2[rs1[gpn[d_loc]] + within1] = s_g
            pv = np.full(NSHP, -1, np.int64)
            pv[:NSH] = gpn
            slot_cg[(c, g)] = (slotv0, slotv2)
            perm_cg[(c, g)] = pv

    # PE permute/replicate matrices (bf16 0/1):
    # Psel8 block k2=(2s+d): col 16k+p (any k) <- row 32s+16d+p
    Psel8 = np.zeros((128, 8 * 128), np.float32)
    for s in range(4):
        for d in range(2):
            k2 = 2 * s + d
            for col in range(128):
                Psel8[32 * s + 16 * d + (col % 16), 128 * k2 + col] = 1.0
    # Wg2r: quad jq replicated: col 16k+p <- Wg2p[:, 16*jq + p]
    Wg2r = np.zeros((H, 4 * 128), np.float32)
    for jq in range(4):
        Wg2r[:, 128 * jq:128 * (jq + 1)] = np.tile(
            Wg2p[:, 16 * jq:16 * (jq + 1)], (1, 8))
    Waq = np.concatenate([Wap[16 * j:16 * (j + 1)] for j in range(4)], 1)

    in_maps = []
    offmaps = None
    for c in range(NCORE):
        lo = c * NSH
        # ---- MP0 stream ----
        eidx, deg, K0 = per_core0[c]
        sub_of_node, j_of_node, gp_of_node = grid_assign[c]
        first = np.concatenate([[0], np.cumsum(deg)[:-1]])
        ea16 = np.zeros((16, slots0), np.float32)
        s_loc = src[eidx] - lo
        within = np.arange(len(eidx)) - first[s_loc]
        node_base = np.zeros(NSH, np.int64)
        for (b, m4, jo, so) in plan0:
            selm = (K0 == b) & (j_of_node >= jo) & (j_of_node < jo + m4)
            node_base[selm] = (so + sub_of_node[selm] * m4 * b
                               + (j_of_node[selm] - jo) * b)
        cols = node_base[s_loc] + within
        ea16[:, cols] = ea[eidx].T

        cnt_n = deg.astype(np.float32)
        k_n = K0.astype(np.float32)
        alpha = 1.0 / np.maximum(cnt_n, 1.0)
        dinv = dinv_all[lo:lo + NSH]
        A128 = np.zeros((128, J), np.float32)
        Bx128 = np.zeros((128, J), np.float32)
        xp = np.zeros((F0, NSH), np.float32)
        xp[:30] = x[lo:lo + NSH].T
        xp = xp[PERM32]
        av = alpha * dinv
        bxv = ((k_n - cnt_n) * alpha)[None, :] * vpad[:, None]
        bxv = (bxv - xp) * dinv[None, :]
        for s_ in range(4):
            selm = sub_of_node == s_
            jj = j_of_node[selm]
            A128[32 * s_:32 * (s_ + 1), jj] = av[selm][None, :].repeat(F0, 0)
            Bx128[32 * s_:32 * (s_ + 1), jj] = bxv[:, selm]
        dinv16p = np.zeros((16, NSHP, 2), np.float32)
        dinv16p[:, :NSH, :] = dinv[None, :, None]
        dinv16p = dinv16p.reshape(16, NSHP * 2)
        dinv128 = np.zeros((128, NSHP), np.float32)
        dinv128[:, :NSH] = dinv[None, :].repeat(128, 0)

        # canonical -> own MP0-grid position, replicated to all 8 blocks
        pv0 = np.full(NSHP, -1, np.int64)
        pv0[:NSH] = gp_of_node
        perm0c = np.zeros((128, NSHP // 16), np.int16)
        for k in range(NCORE):
            perm0c += _wrap16(pv0, 128, base_part=16 * k)

        # ---- gather idx + perm for THIS core: blocks are target cores ----
        idx0 = np.zeros((128, slots1 // 16), np.int16)
        idx2 = np.zeros((128, slots1 // 16), np.int16)
        perm1 = np.zeros((128, NSHP // 16), np.int16)
        for tgt in range(NCORE):
            slotv0, slotv2 = slot_cg[(tgt, c)]
            idx0 += _wrap16(slotv0, 128, base_part=16 * tgt)
            idx2 += _wrap16(slotv2, 128, base_part=16 * tgt)
            perm1 += _wrap16(perm_cg[(tgt, c)], 128, base_part=16 * tgt)

        o16, b16f = _pack([
            ("ea", ea16), ("We1", W["W_e1"]), ("We2p", We2p),
            ("Psel8", Psel8),
            ("Wg1a", Wg1p[0:16]), ("Wg1b", Wg1p[16:32]), ("Wg2r", Wg2r),
            ("Waq", Waq), ("dinv16p", dinv16p), ("bg2j", bg2_j),
        ], bft)
        o32, b32f = _pack([
            ("A128", A128), ("Bx128", Bx128),
            ("be1", W["b_e1"].reshape(H, 1)),
            ("be2q", np.tile(be2p, 4).reshape(H, 1)),
            ("bg1", W["b_g1"].reshape(H, 1)),
            ("ba", W["b_a"].reshape(F2, 1)),
            ("dinv128", dinv128),
        ], np.float32)
        oi, bif = _pack([
            ("idx0", idx0), ("idx2", idx2), ("perm1", perm1),
            ("perm0c", perm0c),
        ], np.int16)
        in_maps.append({"b16": b16f, "b32": b32f, "bi": bif})
        if offmaps is None:
            offmaps = (o16, o32, oi, b16f.shape[1], b32f.shape[1], bif.shape[1])

    plan = dict(J=J, slots0=slots0, plan0=plan0,
                layout1=layout1, NG1=NG1, NG1T=NG1T, slots1=slots1,
                gplan1=gplan1, gplan2=gplan2, offmaps=offmaps)
    fin = (W["W_o"], W["b_o"])
    return in_maps, plan, fin


def build(plan):
    from concourse import bacc, tile
    from concourse.bass import mybir
    dt = mybir.dt
    AF = mybir.ActivationFunctionType
    ALU = mybir.AluOpType
    X = mybir.AxisListType.X

    J, slots0, plan0 = plan["J"], plan["slots0"], plan["plan0"]
    NG1T, slots1 = plan["NG1T"], plan["slots1"]
    gplan1 = plan["gplan1"]
    gplan2 = plan["gplan2"]
    o16, o32, oi, L16, L32, LI = plan["offmaps"]

    nc = bacc.Bacc("TRN2", target_bir_lowering=False, debug=False,
                   num_devices=NCORE)

    b16 = nc.declare_dram_parameter("b16", [1, L16], dt.bfloat16, False)
    b32 = nc.declare_dram_parameter("b32", [1, L32], dt.float32, False)
    bi = nc.declare_dram_parameter("bi", [1, LI], dt.int16, False)
    out = nc.declare_dram_parameter("out", [1, F2], dt.float32, True)

    pin1 = nc.dram_tensor("pin1", [128, NSHP * 2], dt.bfloat16)
    pout1 = nc.dram_tensor("pout1", [16, NSHP * 2], dt.bfloat16)
    pin2h = [nc.dram_tensor(f"pin2{h}", [128, NSHP * 2], dt.bfloat16)
             for h in range(2)]
    pout2h = [nc.dram_tensor(f"pout2{h}", [16, NSHP * 2], dt.bfloat16)
              for h in range(2)]
    RG = [list(range(NCORE))]

    def bsl(blob, offmap, key):
        off, P, n = offmap[key]
        return blob[0:1, off:off + P * n].rearrange("a (p n) -> (a p) n", p=P)

    with tile.TileContext(nc) as tc:
        with tc.tile_pool(name="const", bufs=1) as cpool:
            def load(blob, offmap, key, dtype, pool=None, tag=None):
                off, P, n = offmap[key]
                t = (pool or cpool).tile([P, n], dtype, tag=tag or key)
                nc.sync.dma_start(out=t[:], in_=bsl(blob, offmap, key))
                return t

            We1_s = load(b16, o16, "We1", dt.bfloat16)
            We2p_s = load(b16, o16, "We2p", dt.bfloat16)
            be1_s = load(b32, o32, "be1", dt.float32)
            be2q_s = load(b32, o32, "be2q", dt.float32)
            perm1_s = load(bi, oi, "perm1", dt.int16)

            ea_off = o16["ea"][0]
            ea_ap = b16[0:1, ea_off:ea_off + 16 * slots0].rearrange(
                "a (p s) -> (a p) s", p=16)

            # ---------------- MP0: edge MLP + segment reduce ----------
            vcm = tc.tile_pool(name="vp", bufs=1)
            vp = vcm.__enter__()
            vpair = vp.tile([128, 4 * J, 2], dt.bfloat16, tag="vpair")
            with tc.tile_pool(name="pg", bufs=1) as pg:
                Psel8_s = load(b16, o16, "Psel8", dt.bfloat16, pg)
                sums = pg.tile([128, J], dt.bfloat16, tag="sums")
                nc.vector.memset(sums[:], 0.0)
                with (
                    tc.tile_pool(name="mlp", bufs=3) as mp,
                    tc.tile_pool(name="ps0", bufs=2, space="PSUM") as ps0,
                    tc.tile_pool(name="ps2", bufs=2, space="PSUM") as ps2,
                ):
                    batches = []
                    cur = []
                    cur_n = 0
                    for ch in plan0:
                        n = 4 * ch[0] * ch[1]
                        if cur and cur_n + n > 8192:
                            batches.append((cur, cur_n))
                            cur, cur_n = [], 0
                        cur.append(ch)
                        cur_n += n
                    if cur:
                        batches.append((cur, cur_n))
                    relu_tog = 0
                    for bi_, (chs, bn) in enumerate(batches):
                        b_off = chs[0][3]
                        eat = mp.tile([16, 8192], dt.bfloat16, tag="ea")
                        dma_eng = nc.sync if bi_ % 2 else nc.gpsimd
                        dma_eng.dma_start(
                            out=eat[:, :bn],
                            in_=ea_ap[:, b_off:b_off + bn])
                        ef1 = mp.tile([H, 8192], dt.bfloat16, tag="ef1")
                        for j in range(0, bn, 512):
                            w = min(512, bn - j)
                            pt = ps0.tile([H, 512], dt.float32, tag="ps1")
                            nc.tensor.matmul(out=pt[:, :w], lhsT=We1_s[:],
                                             rhs=eat[:, j:j + w],
                                             start=True, stop=True)
                            relu_tog += 1
                            if relu_tog % 2:
                                nc.scalar.activation(out=ef1[:, j:j + w],
                                                     in_=pt[:, :w],
                                                     func=AF.Relu,
                                                     bias=be1_s[:])
                            else:
                                nc.vector.tensor_scalar(
                                    out=ef1[:, j:j + w], in0=pt[:, :w],
                                    scalar1=be1_s[:], scalar2=0.0,
                                    op0=ALU.add, op1=ALU.max)
                        for (b, m4, jo, so) in chs:
                            sw = m4 * b
                            co = so - b_off
                            pt2 = ps2.tile([H, 512], dt.float32, tag="ps2")
                            for s_ in range(4):
                                nc.tensor.matmul(
                                    out=pt2[32 * s_:32 * (s_ + 1), :sw],
                                    lhsT=We2p_s[:],
                                    rhs=ef1[:, co + s_ * sw:co + (s_ + 1) * sw],
                                    start=True, stop=True,
                                    tile_position=(0, 32 * s_))
                            ef2 = mp.tile([H, 512], dt.bfloat16, tag="ef2")
                            nc.scalar.activation(out=ef2[:, :sw],
                                                 in_=pt2[:, :sw],
                                                 func=AF.Relu,
                                                 bias=be2q_s[:])
                            with nc.allow_low_precision("bf16 run sums"):
                                nc.vector.tensor_reduce(
                                    out=sums[:, jo:jo + m4],
                                    in_=ef2[:, :sw].rearrange(
                                        "p (m b) -> p m b", m=m4),
                                    axis=X, op=ALU.add)
                # v = sums * A - Bx (pads have A=Bx=0 -> exact zeros),
                # then pair table replicated on all 8 blocks via PE permute.
                # Done in column strips so it overlaps the edge-MLP stream.
                A_s = load(b32, o32, "A128", dt.float32, pg)
                Bx_s = load(b32, o32, "Bx128", dt.float32, pg)
                v = pg.tile([128, J], dt.bfloat16, tag="v")
                t0 = pg.tile([128, J], dt.float32, tag="t0")
                with tc.tile_pool(name="psp", bufs=2, space="PSUM") as psp:
                    for jo in range(0, J, 512):
                        w = min(512, J - jo)
                        nc.vector.tensor_tensor(out=t0[:, jo:jo + w],
                                                in0=sums[:, jo:jo + w],
                                                in1=A_s[:, jo:jo + w],
                                                op=ALU.mult)
                        nc.vector.tensor_tensor(out=v[:, jo:jo + w],
                                                in0=t0[:, jo:jo + w],
                                                in1=Bx_s[:, jo:jo + w],
                                                op=ALU.subtract)
                        for s_ in range(4):
                            for d_ in range(2):
                                k2 = 2 * s_ + d_
                                pp = psp.tile([128, 512], dt.float32,
                                              tag="pp")
                                nc.tensor.matmul(
                                    out=pp[:, :w],
                                    lhsT=Psel8_s[:, 128 * k2:128 * (k2 + 1)],
                                    rhs=v[:, jo:jo + w],
                                    start=True, stop=True)
                                nc.scalar.activation(
                                    out=vpair[:, s_ * J + jo:
                                              s_ * J + jo + w, d_],
                                    in_=pp[:, :w], func=AF.Copy)

            # ---------------- MP1: local partials + ReduceScatter -------
            dinv16p_s = load(b16, o16, "dinv16p", dt.bfloat16)
            with tc.tile_pool(name="mpA", bufs=1) as mpA:
                with (
                    tc.tile_pool(name="qgp", bufs=1) as qgp,
                    tc.tile_pool(name="gch", bufs=2) as gchp,
                ):
                    idx0_s = load(bi, oi, "idx0", dt.int16, qgp)
                    qgrid = qgp.tile([128, NG1T, 2], dt.bfloat16,
                                     tag="qgrid")
                    with nc.allow_low_precision("bf16 grid"):
                        for (c_off, c_n, pieces) in gplan1:
                            gch = gchp.tile([128, GCH1, 2], dt.bfloat16,
                                            tag="gch")
                            nc.gpsimd.ap_gather(
                                out_ap=gch[:, :c_n, :], in_ap=vpair[:],
                                idxs_ap=idx0_s[:, c_off // 16:
                                               (c_off + c_n) // 16],
                                channels=128, num_elems=4 * J, d=2,
                                num_idxs=c_n)
                            for (b, m, no, so) in pieces:
                                nc.vector.tensor_reduce(
                                    out=qgrid[:, no:no + m, :],
                                    in_=gch[:, so - c_off:so - c_off + b * m,
                                            :].rearrange(
                                        "p (m b) d -> p m d b", m=m),
                                    axis=X, op=ALU.add)
                    qrs = mpA.tile([128, NSHP, 2], dt.bfloat16, tag="qrs")
                    with nc.allow_low_precision("bf16 permute"):
                        nc.gpsimd.ap_gather(
                            out_ap=qrs[:], in_ap=qgrid[:],
                            idxs_ap=perm1_s[:],
                            channels=128, num_elems=NG1T, d=2,
                            num_idxs=NSHP)
                    qrsf = qrs[:].rearrange("p n d -> p (n d)")
                    nc.sync.dma_start(out=pin1[:, :NSHP],
                                      in_=qrsf[:, :NSHP])
                    nc.scalar.dma_start(out=pin1[:, NSHP:],
                                        in_=qrsf[:, NSHP:])
                nc.gpsimd.collective_compute(
                    "ReduceScatter", ALU.add, replica_groups=RG,
                    ins=[pin1[:]], outs=[pout1[:]])
                qv = cpool.tile([16, NSHP, 2], dt.bfloat16, tag="qv")
                perm0c_s = load(bi, oi, "perm0c", dt.int16, mpA)
                with nc.allow_low_precision("bf16 permute"):
                    nc.gpsimd.ap_gather(
                        out_ap=qv[:], in_ap=vpair[0:16, :, :],
                        idxs_ap=perm0c_s[0:16, :],
                        channels=16, num_elems=4 * J, d=2,
                        num_idxs=NSHP)
            vcm.__exit__(None, None, None)
            p2cm = tc.tile_pool(name="p2p", bufs=1)
            p2p = p2cm.__enter__()
            if True:
                with (
                    tc.tile_pool(name="mpB", bufs=1) as mpB,
                    tc.tile_pool(name="ps1p", bufs=2, space="PSUM") as ps1p,
                ):
                    m1 = mpB.tile([16, NSHP, 2], dt.bfloat16, tag="m1")
                    m1f = m1[:].rearrange("p n d -> p (n d)")
                    nc.sync.dma_start(out=m1f[:, :NSHP],
                                      in_=pout1[:, :NSHP])
                    nc.scalar.dma_start(out=m1f[:, NSHP:],
                                        in_=pout1[:, NSHP:])
                    with nc.allow_low_precision("bf16 m1"):
                        nc.vector.tensor_tensor(
                            out=m1[:], in0=m1[:], in1=qv[:],
                            op=ALU.add)
                        nc.vector.tensor_tensor(
                            out=m1[:].rearrange("p n d -> p (n d)"),
                            in0=m1[:].rearrange("p n d -> p (n d)"),
                            in1=dinv16p_s[:],
                            op=ALU.mult)
                    Wg1a_s = load(b16, o16, "Wg1a", dt.bfloat16, mpB)
                    Wg1b_s = load(b16, o16, "Wg1b", dt.bfloat16, mpB)
                    bg1_s = load(b32, o32, "bg1", dt.float32, mpB)
                    Wg2r_s = load(b16, o16, "Wg2r", dt.bfloat16, mpB)
                    dinv128_s = load(b32, o32, "dinv128", dt.float32, mpB)
                    h1 = mpB.tile([H, NSHP], dt.bfloat16, tag="h1")
                    for o in range(0, NSHP, 512):
                        w = min(512, NSHP - o)
                        hp = ps1p.tile([H, 512], dt.float32, tag="h1p")
                        nc.tensor.matmul(out=hp[:, :w], lhsT=Wg1a_s[:],
                                         rhs=m1[:, o:o + w, 0],
                                         start=True, stop=False)
                        nc.tensor.matmul(out=hp[:, :w], lhsT=Wg1b_s[:],
                                         rhs=m1[:, o:o + w, 1],
                                         start=False, stop=True)
                        nc.scalar.activation(out=h1[:, o:o + w],
                                             in_=hp[:, :w],
                                             func=AF.Relu, bias=bg1_s[:])
                    # h1 *= dinv_src; p2 = Wg2^T h1 replicated, halves
                    nc.vector.tensor_tensor(out=h1[:], in0=h1[:],
                                            in1=dinv128_s[:], op=ALU.mult)
                    h1d = h1
                    p2A = p2p.tile([128, NSHP, 2], dt.bfloat16, tag="p2A")
                    p2B = p2p.tile([128, NSHP, 2], dt.bfloat16, tag="p2B")
                    for o in range(0, NSHP, 512):
                        w = min(512, NSHP - o)
                        for jq in range(4):
                            zp = ps1p.tile([128, 512], dt.float32, tag="zp")
                            nc.tensor.matmul(
                                out=zp[:, :w],
                                lhsT=Wg2r_s[:, 128 * jq:128 * (jq + 1)],
                                rhs=h1d[:, o:o + w],
                                start=True, stop=True)
                            p2h = p2A if jq < 2 else p2B
                            nc.scalar.activation(
                                out=p2h[:, o:o + w, jq % 2],
                                in_=zp[:, :w], func=AF.Copy)

            # ---------------- MP2: two d=2 half passes + ReduceScatter --
            with tc.tile_pool(name="mpA2", bufs=1) as mpA2:
                idx2_s = load(bi, oi, "idx2", dt.int16, mpA2)
                for hh, p2h in enumerate((p2A, p2B)):
                    with (
                        tc.tile_pool(name=f"qg2{hh}", bufs=1) as qgp2,
                        tc.tile_pool(name="gch2", bufs=2) as gchp2,
                    ):
                        qgrid2 = qgp2.tile([128, NG1T, 2], dt.bfloat16,
                                           tag="qgrid2")
                        with nc.allow_low_precision("bf16 grid"):
                            for (c_off, c_n, pieces) in gplan2:
                                gch = gchp2.tile([128, GCH2, 2],
                                                 dt.bfloat16, tag="gch2")
                                nc.gpsimd.ap_gather(
                                    out_ap=gch[:, :c_n, :], in_ap=p2h[:],
                                    idxs_ap=idx2_s[:, c_off // 16:
                                                   (c_off + c_n) // 16],
                                    channels=128, num_elems=NSHP, d=2,
                                    num_idxs=c_n)
                                for (b, m, no, so) in pieces:
                                    nc.vector.tensor_reduce(
                                        out=qgrid2[:, no:no + m, :],
                                        in_=gch[:, so - c_off:
                                                so - c_off + b * m,
                                                :].rearrange(
                                            "p (m b) d -> p m d b", m=m),
                                        axis=X, op=ALU.add)
                        qrs2 = mpA2.tile([128, NSHP, 2], dt.bfloat16,
                                         tag="qrs2")
                        with nc.allow_low_precision("bf16 permute"):
                            nc.gpsimd.ap_gather(
                                out_ap=qrs2[:], in_ap=qgrid2[:],
                                idxs_ap=perm1_s[:],
                                channels=128, num_elems=NG1T, d=2,
                                num_idxs=NSHP)
                        qrs2f = qrs2[:].rearrange("p n d -> p (n d)")
                        nc.sync.dma_start(out=pin2h[hh][:, :NSHP],
                                          in_=qrs2f[:, :NSHP])
                        nc.scalar.dma_start(out=pin2h[hh][:, NSHP:],
                                            in_=qrs2f[:, NSHP:])
                    nc.gpsimd.collective_compute(
                        "ReduceScatter", ALU.add, replica_groups=RG,
                        ins=[pin2h[hh][:]], outs=[pout2h[hh][:]])
            if True:
                with (
                    tc.tile_pool(name="mpB2", bufs=1) as mpB2,
                    tc.tile_pool(name="psa", bufs=2, space="PSUM") as psa,
                    tc.tile_pool(name="sm2", bufs=2) as sm2,
                ):
                    bg2j_s = load(b16, o16, "bg2j", dt.bfloat16, mpB2)
                    ba_s = load(b32, o32, "ba", dt.float32, mpB2)
                    Waq_s = load(b16, o16, "Waq", dt.bfloat16, mpB2)
                    qsum2 = mpB2.tile([16, 2, NSHP, 2], dt.bfloat16,
                                      tag="qsum2")
                    nc.sync.dma_start(
                        out=qsum2[:, 0, :, :].rearrange("p n d -> p (n d)"),
                        in_=pout2h[0][:])
                    nc.scalar.dma_start(
                        out=qsum2[:, 1, :, :].rearrange("p n d -> p (n d)"),
                        in_=pout2h[1][:])
                    asum = mpB2.tile([F2, 1], dt.float32, tag="asum")
                    ab = mpB2.tile([F2, NSHP], dt.bfloat16, tag="ab")
                    h2c = qsum2
                    # half A first (depends only on RS2a -> hides under
                    # RS2b), then half B interleaved with the a-layer
                    def t2_ops(hh, p2h, o, w2):
                        qs = qsum2[:, hh, o:o + w2, :]
                        hv = qs.rearrange("p n d -> p (n d)")
                        nc.vector.tensor_tensor(
                            out=qs, in0=qs,
                            in1=p2h[0:16, o:o + w2, :], op=ALU.add)
                        nc.vector.tensor_tensor(
                            out=hv, in0=hv,
                            in1=dinv16p_s[:, 2 * o:2 * (o + w2)],
                            op=ALU.mult)
                        nc.vector.tensor_tensor(
                            out=qs, in0=qs,
                            in1=bg2j_s[:, 2 * hh:2 * hh + 2]
                            .unsqueeze(1).broadcast_to([16, w2, 2]),
                            op=ALU.add)
                        nc.vector.tensor_scalar_max(
                            out=hv, in0=hv, scalar1=0.0)
                    with nc.allow_low_precision("bf16 h2"):
                        for o in range(0, NSHP, 1024):
                            t2_ops(0, p2A, o, min(1024, NSHP - o))
                    for o in range(0, NSHP, 1024):
                        w2 = min(1024, NSHP - o)
                        with nc.allow_low_precision("bf16 h2"):
                            t2_ops(1, p2B, o, w2)
                        for oo in range(o, o + w2, 512):
                            w = min(512, NSHP - oo)
                            ap_ = psa.tile([F2, 512], dt.float32, tag="ap")
                            for jq in range(4):
                                nc.tensor.matmul(
                                    out=ap_[:, :w],
                                    lhsT=Waq_s[:, 64 * jq:64 * (jq + 1)],
                                    rhs=h2c[:, jq // 2, oo:oo + w, jq % 2],
                                    start=(jq == 0), stop=(jq == 3))
                            nc.scalar.activation(out=ab[:, oo:oo + w],
                                                 in_=ap_[:, :w],
                                                 func=AF.Relu, bias=ba_s[:])
                    nc.vector.tensor_reduce(out=asum[:], in_=ab[:, :NSH],
                                            axis=X, op=ALU.add)
                    nc.sync.dma_start(out=out[:].rearrange("o p -> p o"),
                                      in_=asum[:])
            p2cm.__exit__(None, None, None)
    nc.compile()
    return nc


def kernel(trace=False, **inputs):
    from concourse.bass_utils import run_bass_kernel_spmd
    in_maps, plan, fin = host_prep(inputs)
    nc = build(plan)
    res = run_bass_kernel_spmd(nc, in_maps, core_ids=list(range(NCORE)),
                               trace=trace)
    Wo, bo = fin
    tot = np.zeros(F2, np.float64)
    for c in range(NCORE):
        tot += np.asarray(res.results[c]["out"], np.float32).reshape(F2)
    y = (tot / N) @ Wo.astype(np.float64) + bo.astype(np.float64)
    o = y.astype(np.float32)
    if trace:
        return o, res
    return o
